# revision 23
# baseline (speedup 1.0000x reference)
"""Bass/Trainium2 kernel for the bidirectional 2-layer GRU PoS model.

Sharding: data-parallel over batch. Each of the 8 cores processes 8 of the
64 sequences end-to-end (embed gather -> W1 -> 2x BiGRU -> W2+gelu -> W3).
Weights are replicated; no collectives.

Layout convention on-chip: "transposed" / feature-on-partitions. Tokens are
ordered time-major: token column = t*BC + b. This makes the per-timestep
scan slices contiguous ([128, BC] blocks) and lets the GRU state feed the
recurrent matmuls (contraction over the hidden dim = partitions) directly.
"""

import os

os.environ.setdefault("MYCRO_LOCAL_CACHE", "1")

import numpy as np

import concourse.bass as bass
import concourse.mybir as mybir
import concourse.tile as tile
from concourse import bacc
from concourse.bass import IndirectOffsetOnAxis
from concourse import bass_utils
from concourse.masks import make_identity

F32 = mybir.dt.float32
F32R = mybir.dt.float32r
I32 = mybir.dt.int32
AF = mybir.ActivationFunctionType
ALU = mybir.AluOpType

VOCAB, EMBED, HID, OUTD = 50000, 256, 256, 50
LAYERS = 2
B_FULL, S_FULL = 64, 512
NCORES = 8
BC = B_FULL // NCORES  # 8 sequences per core
H = HID // 2  # 128 per-direction hidden

# dtype knobs: f32r = native single-pass fp32 on the PE (fast), f32 = 2-pass.
BULK_F32R = False
SCAN_F32R = False


def _cast(ap, on):
    return ap.bitcast(F32R) if on else ap


def build_nc(S=S_FULL, gelu=AF.Gelu, upto="all"):
    LEVELS = {"prep": 0, "embed": 1, "xp": 2, "scan": 3, "all": 4}
    LVL = LEVELS[upto]
    NTOK = BC * S
    G = NTOK // 128  # token tiles for embedding gather
    NT = 512 if NTOK % 512 == 0 else NTOK  # matmul n-tile (tokens)
    NNT = NTOK // NT

    nc = bacc.Bacc("TRN2", target_bir_lowering=False, debug=False,
                   num_devices=NCORES)

    # ---- DRAM I/O ----
    d_xcols = nc.dram_tensor("x_cols", [128, G], I32, kind="ExternalInput").ap()
    d_embed = nc.dram_tensor("embed", [VOCAB, EMBED], F32, kind="ExternalInput").ap()
    d_w1 = nc.dram_tensor("W1", [HID, EMBED], F32, kind="ExternalInput").ap()
    d_b1 = nc.dram_tensor("b1", [HID], F32, kind="ExternalInput").ap()
    d_wih = nc.dram_tensor("gru_Wih", [LAYERS, 2, 3 * H, HID], F32, kind="ExternalInput").ap()
    d_whh = nc.dram_tensor("gru_Whh", [LAYERS, 2, 3 * H, H], F32, kind="ExternalInput").ap()
    d_bih = nc.dram_tensor("gru_bih", [LAYERS, 2, 3 * H], F32, kind="ExternalInput").ap()
    d_bhh = nc.dram_tensor("gru_bhh", [LAYERS, 2, 3 * H], F32, kind="ExternalInput").ap()
    d_w2 = nc.dram_tensor("W2", [HID, HID], F32, kind="ExternalInput").ap()
    d_b2 = nc.dram_tensor("b2", [HID], F32, kind="ExternalInput").ap()
    d_w3 = nc.dram_tensor("W3", [OUTD, HID], F32, kind="ExternalInput").ap()
    d_b3 = nc.dram_tensor("b3", [OUTD], F32, kind="ExternalInput").ap()
    d_y = nc.dram_tensor("y", [NTOK, OUTD], F32, kind="ExternalOutput").ap()

    # ---- persistent SBUF ----
    def sb(name, shape, dt=F32):
        return nc.alloc_sbuf_tensor(name, list(shape), dt).ap()

    # big [128, NTOK] arena:
    #  0,1: hT (current layer input, transposed; holds h1 then h2)
    #  2..7: xp buffers (r,z,n) x (fwd, bwd) for current layer
    #  8,9: ET (embedding transposed) -> later reused as outF / outB
    #  2,3 reused at tail for out2T
    ar = [sb(f"ar{i}", [128, NTOK]) for i in range(10)]
    hT = [ar[0], ar[1]]
    xpF = [ar[2], ar[3], ar[4]]  # r, z, n (fwd)
    xpB = [ar[5], ar[6], ar[7]]  # r, z, n (bwd)
    ET = [ar[8], ar[9]]
    outF, outB = ar[8], ar[9]
    out2T = [ar[2], ar[3]]

    ident = sb("ident", [128, 128])
    idx_sb = sb("idx", [128, G], I32)
    ones_row = sb("ones_row", [1, 128])
    b3_row = sb("b3_row", [1, 64])
    zeros_bc = sb("zeros_bc", [128, BC])
    bhh_n_stage = sb("bhh_n_stage", [128, 2 * BC])

    # transposed weights
    w1T = [sb(f"w1T{k}", [128, HID]) for k in range(2)]
    w2T = [sb(f"w2T{k}", [128, HID]) for k in range(2)]
    w3T = [sb(f"w3T{k}", [128, OUTD]) for k in range(2)]
    wihT = [[[sb(f"wihT_{l}_{d}_{k}", [128, 3 * H]) for k in range(2)]
             for d in range(2)] for l in range(LAYERS)]
    whhT = [[[sb(f"whhT_{l}_{d}_{g}", [128, H]) for g in range(3)]
             for d in range(2)] for l in range(LAYERS)]

    # per-partition bias columns [128, 1]
    b1c = [sb(f"b1c{m}", [128, 1]) for m in range(2)]
    b2c = [sb(f"b2c{m}", [128, 1]) for m in range(2)]
    # per (layer, dir): rz-folded (bih+bhh) for gates r,z ; bih_n ; bhh_n
    brz = [[[sb(f"brz_{l}_{d}_{g}", [128, 1]) for g in range(2)]
            for d in range(2)] for l in range(LAYERS)]
    bihn = [[sb(f"bihn_{l}_{d}", [128, 1]) for d in range(2)] for l in range(LAYERS)]
    bhhn = [[sb(f"bhhn_{l}_{d}", [128, 1]) for d in range(2)] for l in range(LAYERS)]

    with tile.TileContext(nc) as tc:
        # all gpsimd-engine prep first, then one PE op consuming ident so
        # later transpose-matmuls (which have a single sync-wait slot) only
        # ever need to wait on their DMA.
        nc.gpsimd.memset(ones_row[:, :], 1.0)
        nc.gpsimd.memset(zeros_bc[:, :], 0.0)
        nc.gpsimd.memset(b3_row[:, :], 0.0)
        make_identity(nc, ident[:, :])
        nc.sync.dma_start(out=idx_sb[:, :], in_=d_xcols[:, :])

        # ---------- weight prep ----------
        with tc.tile_pool(name="wprep", bufs=8) as wp, \
             tc.tile_pool(name="bstage", bufs=1) as bp, \
             tc.tile_pool(name="wprep_ps", bufs=4, space="PSUM") as wps:
            # transpose via a NORMAL matmul (out = lhsT.T @ I): the
            # is_transpose path only has one sync-wait slot in walrus codegen.
            def mm_transpose(out, in_, rsz, csz):
                nc.tensor.matmul(out=out, lhsT=in_, rhs=ident[:rsz, :rsz],
                                 start=True, stop=True)

            dummy_ps = wps.tile([128, 128], F32, tag="wps")
            mm_transpose(dummy_ps[:, :], ident[:, :], 128, 128)

            def load_T(dst_tiles, wa, R, C):
                # dst_tiles[cb][0:csz, 0:R] = wa.T block-columns
                for cb in range(len(dst_tiles)):
                    csz = min(128, C - cb * 128)
                    for rb in range((R + 127) // 128):
                        rsz = min(128, R - rb * 128)
                        tmp = wp.tile([128, 128], F32, tag="wtmp")
                        nc.sync.dma_start(
                            out=tmp[:rsz, :csz],
                            in_=wa[rb * 128:rb * 128 + rsz, cb * 128:cb * 128 + csz])
                        ps = wps.tile([128, 128], F32, tag="wps")
                        mm_transpose(ps[:csz, :rsz], tmp[:rsz, :csz], rsz, csz)
                        nc.vector.tensor_copy(
                            out=dst_tiles[cb][:csz, rb * 128:rb * 128 + rsz],
                            in_=ps[:csz, :rsz])

            load_T(w1T, d_w1, HID, EMBED)
            load_T(w2T, d_w2, HID, HID)
            load_T(w3T, d_w3, OUTD, HID)
            for l in range(LAYERS):
                for d in range(2):
                    load_T(wihT[l][d], d_wih[l, d], 3 * H, HID)
                    # Whh: [384, 128] -> single column block, but split by gate
                    # into three [128,128] stationaries.
                    for g in range(3):
                        tmp = wp.tile([128, 128], F32, tag="wtmp")
                        nc.sync.dma_start(out=tmp[:, :],
                                          in_=d_whh[l, d][g * H:(g + 1) * H, :])
                        ps = wps.tile([128, 128], F32, tag="wps")
                        mm_transpose(ps[:, :], tmp[:, :], 128, 128)
                        nc.vector.tensor_copy(out=whhT[l][d][g][:, :], in_=ps[:, :])

            # ---------- bias prep ----------
            # stage all bias vectors on partition 0, then "columnize" each
            # 128-chunk to a [128,1] tile via a K=1 matmul with ones.
            stages = {}
            for name, ap_, n in (("b1", d_b1, HID), ("b2", d_b2, HID),
                                 ("b3", d_b3, OUTD),
                                 ("bih", d_bih.flatten(), LAYERS * 2 * 3 * H),
                                 ("bhh", d_bhh.flatten(), LAYERS * 2 * 3 * H)):
                st = bp.tile([1, n], F32, tag=f"bstage_{name}")
                nc.sync.dma_start(out=st[0:1, :], in_=ap_.unsqueeze(0))
                stages[name] = st

            nc.vector.tensor_copy(out=b3_row[0:1, :OUTD], in_=stages["b3"][0:1, :OUTD])

            def columnize(dst, stage_name, src_off, n=128):
                stage = stages[stage_name]
                ps = wps.tile([128, 1], F32, tag="bcol")
                nc.tensor.matmul(out=ps[:n, :], lhsT=stage[0:1, src_off:src_off + n],
                                 rhs=ones_row[0:1, 0:1], start=True, stop=True)
                nc.vector.tensor_copy(out=dst[:n, :], in_=ps[:n, :])

            for m in range(2):
                columnize(b1c[m], "b1", m * 128)
                columnize(b2c[m], "b2", m * 128)
            for l in range(LAYERS):
                for d in range(2):
                    base = (l * 2 + d) * 3 * H
                    for g in range(2):  # r, z -> folded bih+bhh
                        tmp_a = bp.tile([128, 1], F32, tag="btmp_a")
                        tmp_b = bp.tile([128, 1], F32, tag="btmp_b")
                        ps = wps.tile([128, 1], F32, tag="bcol")
                        nc.tensor.matmul(out=ps[:, :],
                                         lhsT=stages["bih"][0:1, base + g * H:base + (g + 1) * H],
                                         rhs=ones_row[0:1, 0:1], start=True, stop=True)
                        nc.vector.tensor_copy(out=tmp_a[:, :], in_=ps[:, :])
                        ps2 = wps.tile([128, 1], F32, tag="bcol")
                        nc.tensor.matmul(out=ps2[:, :],
                                         lhsT=stages["bhh"][0:1, base + g * H:base + (g + 1) * H],
                                         rhs=ones_row[0:1, 0:1], start=True, stop=True)
                        nc.vector.tensor_copy(out=tmp_b[:, :], in_=ps2[:, :])
                        nc.vector.tensor_add(out=brz[l][d][g][:, :],
                                             in0=tmp_a[:, :], in1=tmp_b[:, :])
                    columnize(bihn[l][d], "bih", base + 2 * H)
                    columnize(bhhn[l][d], "bhh", base + 2 * H)

        # ---------- embedding gather + transpose + W1 ----------
        if LVL >= 1:
          with tc.tile_pool(name="emb", bufs=6) as ep, \
             tc.tile_pool(name="emb_ps", bufs=4, space="PSUM") as eps:
            for g in range(G):
                et = ep.tile([128, EMBED], F32, tag="etile")
                nc.gpsimd.indirect_dma_start(
                    out=et[:, :], out_offset=None, in_=d_embed[:, :],
                    in_offset=IndirectOffsetOnAxis(ap=idx_sb[:, g:g + 1], axis=0))
                for fb in range(2):
                    ps = eps.tile([128, 128], F32, tag="eps")
                    nc.tensor.matmul(out=ps[:, :], lhsT=et[:, fb * 128:(fb + 1) * 128],
                                     rhs=ident[:, :], start=True, stop=True)
                    nc.vector.tensor_copy(out=ET[fb][:, g * 128:(g + 1) * 128],
                                          in_=ps[:, :])

            # h1T[m] = W1[m-block] @ ET + b1   (feature-major)
            for m in range(2):
                for n in range(NNT):
                    ps = eps.tile([128, NT], F32, tag="mmps")
                    for k in range(2):
                        nc.tensor.matmul(
                            out=ps[:, :],
                            lhsT=_cast(w1T[k][:, m * 128:(m + 1) * 128], BULK_F32R),
                            rhs=_cast(ET[k][:, n * NT:(n + 1) * NT], BULK_F32R),
                            start=(k == 0), stop=(k == 1))
                    nc.scalar.activation(out=hT[m][:, n * NT:(n + 1) * NT], in_=ps[:, :],
                                         func=AF.Identity, bias=b1c[m][:, :], scale=1.0)

        # ---------- layers ----------
        for l in range(LAYERS if LVL >= 2 else 0):
            # input projections xp[dir][gate] = Wih_ld[gate] @ hT (+ biases)
            with tc.tile_pool(name=f"xp{l}_ps", bufs=6, space="PSUM") as xps:
                for d in range(2):
                    xp = xpF if d == 0 else xpB
                    for m in range(3):
                        bias = brz[l][d][m] if m < 2 else bihn[l][d]
                        for n in range(NNT):
                            ps = xps.tile([128, NT], F32, tag="xpps")
                            for k in range(2):
                                nc.tensor.matmul(
                                    out=ps[:, :],
                                    lhsT=_cast(wihT[l][d][k][:, m * 128:(m + 1) * 128], BULK_F32R),
                                    rhs=_cast(hT[k][:, n * NT:(n + 1) * NT], BULK_F32R),
                                    start=(k == 0), stop=(k == 1))
                            nc.scalar.activation(out=xp[m][:, n * NT:(n + 1) * NT],
                                                 in_=ps[:, :], func=AF.Identity,
                                                 bias=bias[:, :], scale=1.0)

            # bhh_n broadcast stage for the n-gate PSUM preload
            nc.vector.tensor_scalar_add(out=bhh_n_stage[:, 0:BC], in0=zeros_bc[:, :],
                                        scalar1=bhhn[l][0][:, :])
            nc.vector.tensor_scalar_add(out=bhh_n_stage[:, BC:2 * BC], in0=zeros_bc[:, :],
                                        scalar1=bhhn[l][1][:, :])

            # ---------- the scan ----------
            W = whhT[l]
            with tc.tile_pool(name=f"scan{l}_rz", bufs=3, space="PSUM") as prz, \
                 tc.tile_pool(name=f"scan{l}_n", bufs=3, space="PSUM") as pn, \
                 tc.tile_pool(name=f"scan{l}_sb", bufs=3) as psb:
                for t in range(S if LVL >= 3 else 0):
                    u = S - 1 - t  # bwd position
                    hF = outF[:, (t - 1) * BC:t * BC] if t > 0 else zeros_bc[:, :]
                    hB = outB[:, (u + 1) * BC:(u + 2) * BC] if t > 0 else zeros_bc[:, :]

                    rz = prz.tile([128, 4 * BC], F32, tag="rz")
                    hn = pn.tile([128, 2 * BC], F32, tag="hn")

                    # PSUM preload of xp (+bhh_n) via identity matmuls
                    idw = _cast(ident[:, :], SCAN_F32R)
                    nc.tensor.matmul(out=rz[:, 0:BC], lhsT=idw,
                                     rhs=_cast(xpF[0][:, t * BC:(t + 1) * BC], SCAN_F32R),
                                     start=True, stop=False, skip_group_check=True)
                    nc.tensor.matmul(out=rz[:, BC:2 * BC], lhsT=idw,
                                     rhs=_cast(xpB[0][:, u * BC:(u + 1) * BC], SCAN_F32R),
                                     start=False, stop=False, skip_group_check=True)
                    nc.tensor.matmul(out=rz[:, 2 * BC:3 * BC], lhsT=idw,
                                     rhs=_cast(xpF[1][:, t * BC:(t + 1) * BC], SCAN_F32R),
                                     start=False, stop=False, skip_group_check=True)
                    nc.tensor.matmul(out=rz[:, 3 * BC:4 * BC], lhsT=idw,
                                     rhs=_cast(xpB[1][:, u * BC:(u + 1) * BC], SCAN_F32R),
                                     start=False, stop=False, skip_group_check=True)
                    nc.tensor.matmul(out=hn[:, :], lhsT=idw,
                                     rhs=_cast(bhh_n_stage[:, :], SCAN_F32R),
                                     start=True, stop=False, skip_group_check=True)

                    # gate matmuls accumulate Whh @ h
                    hFc = _cast(hF, SCAN_F32R)
                    hBc = _cast(hB, SCAN_F32R)
                    nc.tensor.matmul(out=rz[:, 0:BC], lhsT=_cast(W[0][0][:, :], SCAN_F32R),
                                     rhs=hFc, start=False, stop=False, skip_group_check=True)
                    nc.tensor.matmul(out=rz[:, BC:2 * BC], lhsT=_cast(W[1][0][:, :], SCAN_F32R),
                                     rhs=hBc, start=False, stop=False, skip_group_check=True)
                    nc.tensor.matmul(out=rz[:, 2 * BC:3 * BC], lhsT=_cast(W[0][1][:, :], SCAN_F32R),
                                     rhs=hFc, start=False, stop=True, skip_group_check=True)
                    nc.tensor.matmul(out=rz[:, 3 * BC:4 * BC], lhsT=_cast(W[1][1][:, :], SCAN_F32R),
                                     rhs=hBc, start=False, stop=True, skip_group_check=True)
                    nc.tensor.matmul(out=hn[:, 0:BC], lhsT=_cast(W[0][2][:, :], SCAN_F32R),
                                     rhs=hFc, start=False, stop=False, skip_group_check=True)
                    nc.tensor.matmul(out=hn[:, BC:2 * BC], lhsT=_cast(W[1][2][:, :], SCAN_F32R),
                                     rhs=hBc, start=False, stop=True, skip_group_check=True)

                    # sigmoid over r|z for both dirs, then gate algebra
                    rz_sb = psb.tile([128, 4 * BC], F32, tag="rz_sb")
                    nc.scalar.activation(out=rz_sb[:, :], in_=rz[:, :], func=AF.Sigmoid)

                    t1 = psb.tile([128, 2 * BC], F32, tag="t1")
                    nc.vector.tensor_mul(out=t1[:, :], in0=rz_sb[:, 0:2 * BC], in1=hn[:, :])
                    npre = psb.tile([128, 2 * BC], F32, tag="npre")
                    nc.vector.tensor_add(out=npre[:, 0:BC], in0=t1[:, 0:BC],
                                         in1=xpF[2][:, t * BC:(t + 1) * BC])
                    nc.vector.tensor_add(out=npre[:, BC:2 * BC], in0=t1[:, BC:2 * BC],
                                         in1=xpB[2][:, u * BC:(u + 1) * BC])

                    omz = psb.tile([128, 2 * BC], F32, tag="omz")
                    nc.vector.tensor_scalar(out=omz[:, :], in0=rz_sb[:, 2 * BC:4 * BC],
                                            scalar1=-1.0, scalar2=1.0,
                                            op0=ALU.mult, op1=ALU.add)
                    p = psb.tile([128, 2 * BC], F32, tag="p")
                    nc.vector.tensor_mul(out=p[:, 0:BC], in0=rz_sb[:, 2 * BC:3 * BC], in1=hF)
                    nc.vector.tensor_mul(out=p[:, BC:2 * BC], in0=rz_sb[:, 3 * BC:4 * BC], in1=hB)

                    n_sb = psb.tile([128, 2 * BC], F32, tag="n_sb")
                    nc.scalar.activation(out=n_sb[:, :], in_=npre[:, :], func=AF.Tanh)

                    t2 = psb.tile([128, 2 * BC], F32, tag="t2")
                    nc.vector.tensor_mul(out=t2[:, :], in0=omz[:, :], in1=n_sb[:, :])
                    nc.vector.tensor_add(out=outF[:, t * BC:(t + 1) * BC],
                                         in0=t2[:, 0:BC], in1=p[:, 0:BC])
                    nc.vector.tensor_add(out=outB[:, u * BC:(u + 1) * BC],
                                         in0=t2[:, BC:2 * BC], in1=p[:, BC:2 * BC])

            # residual + concat: hT <- [outF, outB] + hT
            CH = 2048 if NTOK % 2048 == 0 else NTOK
            for c in range(NTOK // CH if LVL >= 3 else 0):
                sl = slice(c * CH, (c + 1) * CH)
                nc.vector.tensor_add(out=hT[0][:, sl], in0=hT[0][:, sl], in1=outF[:, sl])
                nc.vector.tensor_add(out=hT[1][:, sl], in0=hT[1][:, sl], in1=outB[:, sl])

        # ---------- tail: W2 + gelu, W3 + b3, output ----------
        if LVL >= 4:
          with tc.tile_pool(name="tail", bufs=4) as tp, \
             tc.tile_pool(name="tail_ps", bufs=4, space="PSUM") as tps:
            for m in range(2):
                for n in range(NNT):
                    ps = tps.tile([128, NT], F32, tag="w2ps")
                    for k in range(2):
                        nc.tensor.matmul(
                            out=ps[:, :],
                            lhsT=_cast(w2T[k][:, m * 128:(m + 1) * 128], BULK_F32R),
                            rhs=_cast(hT[k][:, n * NT:(n + 1) * NT], BULK_F32R),
                            start=(k == 0), stop=(k == 1))
                    nc.scalar.activation(out=out2T[m][:, n * NT:(n + 1) * NT], in_=ps[:, :],
                                         func=gelu, bias=b2c[m][:, :], scale=1.0)

            for mt in range(NTOK // 128):
                ps = tps.tile([128, OUTD], F32, tag="w3ps")
                # bias via K=1 ones matmul, then accumulate the two k-blocks
                nc.tensor.matmul(out=ps[:, :], lhsT=ones_row[0:1, :],
                                 rhs=b3_row[0:1, :OUTD], start=True, stop=False,
                                 skip_group_check=True)
                for k in range(2):
                    nc.tensor.matmul(
                        out=ps[:, :],
                        lhsT=_cast(out2T[k][:, mt * 128:(mt + 1) * 128], BULK_F32R),
                        rhs=_cast(w3T[k][:, :OUTD], BULK_F32R),
                        start=False, stop=(k == 1), skip_group_check=True)
                yt = tp.tile([128, OUTD], F32, tag="ytile")
                nc.vector.tensor_copy(out=yt[:, :], in_=ps[:, :])
                nc.sync.dma_start(out=d_y[mt * 128:(mt + 1) * 128, :], in_=yt[:, :])

    nc.compile()
    return nc


# ---------------- host wrapper ----------------

_NC_CACHE = {}


def _get_nc(S):
    if S not in _NC_CACHE:
        _NC_CACHE[S] = build_nc(S)
    return _NC_CACHE[S]


def make_in_maps(x, embed, W1, b1, gru_Wih, gru_Whh, gru_bih, gru_bhh,
                 W2, b2, W3, b3, S=S_FULL):
    f = lambda a: np.ascontiguousarray(np.asarray(a), dtype=np.float32)
    x = np.asarray(x)
    G = BC * S // 128
    common = dict(embed=f(embed), W1=f(W1), b1=f(b1), gru_Wih=f(gru_Wih),
                  gru_Whh=f(gru_Whh), gru_bih=f(gru_bih), gru_bhh=f(gru_bhh),
                  W2=f(W2), b2=f(b2), W3=f(W3), b3=f(b3))
    in_maps = []
    for c in range(NCORES):
        xs = x[c * BC:(c + 1) * BC, :S].astype(np.int32)  # [BC, S]
        toks = np.ascontiguousarray(xs.T).reshape(-1)  # time-major [S*BC]
        x_cols = np.ascontiguousarray(toks.reshape(G, 128).T)  # [128, G]
        in_maps.append(dict(x_cols=x_cols, **common))
    return in_maps


def run(inputs, S=S_FULL, trace=False):
    nc = _get_nc(S)
    in_maps = make_in_maps(**inputs, S=S)
    res = bass_utils.run_bass_kernel_spmd(nc, in_maps, core_ids=list(range(NCORES)),
                                          trace=trace)
    outs = []
    for c in range(NCORES):
        y_tm = res.results[c]["y"]  # [S*BC, OUTD] time-major
        outs.append(y_tm.reshape(S, BC, OUTD).transpose(1, 0, 2))
    return np.concatenate(outs, axis=0), res


def kernel(**inputs):
    out, _ = run(inputs, S=S_FULL, trace=False)
    return out


# revision 25
# speedup vs baseline: 1.6377x; 1.6377x over previous
"""Bass/Trainium2 kernel for the bidirectional 2-layer GRU PoS model.

Sharding: data-parallel over batch. Each of the 8 cores processes 8 of the
64 sequences end-to-end (embed gather -> W1 -> 2x BiGRU -> W2+gelu -> W3).
Weights are replicated; no collectives.

Layout convention on-chip: "transposed" / feature-on-partitions. Tokens are
ordered time-major: token column = t*BC + b. This makes the per-timestep
scan slices contiguous ([128, BC] blocks) and lets the GRU state feed the
recurrent matmuls (contraction over the hidden dim = partitions) directly.
"""

import os

os.environ.setdefault("MYCRO_LOCAL_CACHE", "1")

import numpy as np

import concourse.bass as bass
import concourse.mybir as mybir
import concourse.tile as tile
from concourse import bacc
from concourse.bass import IndirectOffsetOnAxis
from concourse import bass_utils
from concourse.masks import make_identity

F32 = mybir.dt.float32
F32R = mybir.dt.float32r
I32 = mybir.dt.int32
AF = mybir.ActivationFunctionType
ALU = mybir.AluOpType

VOCAB, EMBED, HID, OUTD = 50000, 256, 256, 50
LAYERS = 2
B_FULL, S_FULL = 64, 512
NCORES = 8
BC = B_FULL // NCORES  # 8 sequences per core
H = HID // 2  # 128 per-direction hidden

# dtype knobs: f32r = native single-pass fp32 on the PE (fast), f32 = 2-pass.
BULK_F32R = True
SCAN_F32R = True


def _cast(ap, on):
    return ap.bitcast(F32R) if on else ap


def build_nc(S=S_FULL, gelu=AF.Gelu, upto="all"):
    LEVELS = {"prep": 0, "embed": 1, "xp": 2, "scan": 3, "all": 4}
    LVL = LEVELS[upto]
    NTOK = BC * S
    G = NTOK // 128  # token tiles for embedding gather
    NT = 512 if NTOK % 512 == 0 else NTOK  # matmul n-tile (tokens)
    NNT = NTOK // NT

    nc = bacc.Bacc("TRN2", target_bir_lowering=False, debug=False,
                   num_devices=NCORES)

    # ---- DRAM I/O ----
    d_xcols = nc.dram_tensor("x_cols", [128, G], I32, kind="ExternalInput").ap()
    d_embed = nc.dram_tensor("embed", [VOCAB, EMBED], F32, kind="ExternalInput").ap()
    d_w1 = nc.dram_tensor("W1", [HID, EMBED], F32, kind="ExternalInput").ap()
    d_b1 = nc.dram_tensor("b1", [HID], F32, kind="ExternalInput").ap()
    d_wih = nc.dram_tensor("gru_Wih", [LAYERS, 2, 3 * H, HID], F32, kind="ExternalInput").ap()
    d_whh = nc.dram_tensor("gru_Whh", [LAYERS, 2, 3 * H, H], F32, kind="ExternalInput").ap()
    d_bih = nc.dram_tensor("gru_bih", [LAYERS, 2, 3 * H], F32, kind="ExternalInput").ap()
    d_bhh = nc.dram_tensor("gru_bhh", [LAYERS, 2, 3 * H], F32, kind="ExternalInput").ap()
    d_w2 = nc.dram_tensor("W2", [HID, HID], F32, kind="ExternalInput").ap()
    d_b2 = nc.dram_tensor("b2", [HID], F32, kind="ExternalInput").ap()
    d_w3 = nc.dram_tensor("W3", [OUTD, HID], F32, kind="ExternalInput").ap()
    d_b3 = nc.dram_tensor("b3", [OUTD], F32, kind="ExternalInput").ap()
    d_y = nc.dram_tensor("y", [NTOK, OUTD], F32, kind="ExternalOutput").ap()

    # ---- persistent SBUF ----
    def sb(name, shape, dt=F32):
        return nc.alloc_sbuf_tensor(name, list(shape), dt).ap()

    # big [128, NTOK] arena:
    #  0,1: hT (current layer input, transposed; holds h1 then h2)
    #  2..7: xp buffers (r,z,n) x (fwd, bwd) for current layer
    #  8,9: ET (embedding transposed) -> later reused as outF / outB
    #  2,3 reused at tail for out2T
    ar = [sb(f"ar{i}", [128, NTOK], F32R) for i in range(10)]
    hT = [ar[0], ar[1]]
    xpF = [ar[2], ar[3], ar[4]]  # r, z, n (fwd)
    xpB = [ar[5], ar[6], ar[7]]  # r, z, n (bwd)
    ET = [ar[8], ar[9]]
    outF, outB = ar[8], ar[9]
    out2T = [ar[2], ar[3]]

    ident = sb("ident", [128, 128])
    ident_r = sb("ident_r", [128, 128], F32R)
    idx_sb = sb("idx", [128, G], I32)
    ones_row = sb("ones_row", [1, 128])
    b3_row = sb("b3_row", [1, 64])
    zeros_bc = sb("zeros_bc", [128, BC], F32R)
    bhh_n_stage = sb("bhh_n_stage", [128, 2 * BC], F32R)

    # transposed weights
    w1T = [sb(f"w1T{k}", [128, HID], F32R) for k in range(2)]
    w2T = [sb(f"w2T{k}", [128, HID], F32R) for k in range(2)]
    w3T = [sb(f"w3T{k}", [128, OUTD], F32R) for k in range(2)]
    wihT = [[[sb(f"wihT_{l}_{d}_{k}", [128, 3 * H], F32R) for k in range(2)]
             for d in range(2)] for l in range(LAYERS)]
    whhT = [[[sb(f"whhT_{l}_{d}_{g}", [128, H], F32R) for g in range(3)]
             for d in range(2)] for l in range(LAYERS)]

    # per-partition bias columns [128, 1]
    b1c = [sb(f"b1c{m}", [128, 1]) for m in range(2)]
    b2c = [sb(f"b2c{m}", [128, 1]) for m in range(2)]
    # per (layer, dir): rz-folded (bih+bhh) for gates r,z ; bih_n ; bhh_n
    brz = [[[sb(f"brz_{l}_{d}_{g}", [128, 1]) for g in range(2)]
            for d in range(2)] for l in range(LAYERS)]
    bihn = [[sb(f"bihn_{l}_{d}", [128, 1]) for d in range(2)] for l in range(LAYERS)]
    bhhn = [[sb(f"bhhn_{l}_{d}", [128, 1]) for d in range(2)] for l in range(LAYERS)]

    with tile.TileContext(nc) as tc:
        # all gpsimd-engine prep first, then one PE op consuming ident so
        # later transpose-matmuls (which have a single sync-wait slot) only
        # ever need to wait on their DMA.
        nc.gpsimd.memset(ones_row[:, :], 1.0)
        nc.gpsimd.memset(zeros_bc[:, :].bitcast(F32), 0.0)
        nc.gpsimd.memset(b3_row[:, :], 0.0)
        make_identity(nc, ident[:, :])
        nc.vector.tensor_copy(out=ident_r[:, :], in_=ident[:, :])
        nc.sync.dma_start(out=idx_sb[:, :], in_=d_xcols[:, :])

        # ---------- weight prep ----------
        with tc.tile_pool(name="wprep", bufs=8) as wp, \
             tc.tile_pool(name="bstage", bufs=1) as bp, \
             tc.tile_pool(name="wprep_ps", bufs=4, space="PSUM") as wps:
            # transpose via a NORMAL matmul (out = lhsT.T @ I): the
            # is_transpose path only has one sync-wait slot in walrus codegen.
            def mm_transpose(out, in_, rsz, csz):
                nc.tensor.matmul(out=out, lhsT=in_, rhs=ident[:rsz, :rsz],
                                 start=True, stop=True)

            dummy_ps = wps.tile([128, 128], F32, tag="wps")
            mm_transpose(dummy_ps[:, :], ident[:, :], 128, 128)

            def load_T(dst_tiles, wa, R, C):
                # dst_tiles[cb][0:csz, 0:R] = wa.T block-columns
                for cb in range(len(dst_tiles)):
                    csz = min(128, C - cb * 128)
                    for rb in range((R + 127) // 128):
                        rsz = min(128, R - rb * 128)
                        tmp = wp.tile([128, 128], F32, tag="wtmp")
                        nc.sync.dma_start(
                            out=tmp[:rsz, :csz],
                            in_=wa[rb * 128:rb * 128 + rsz, cb * 128:cb * 128 + csz])
                        ps = wps.tile([128, 128], F32, tag="wps")
                        mm_transpose(ps[:csz, :rsz], tmp[:rsz, :csz], rsz, csz)
                        nc.vector.tensor_copy(
                            out=dst_tiles[cb][:csz, rb * 128:rb * 128 + rsz],
                            in_=ps[:csz, :rsz])

            load_T(w1T, d_w1, HID, EMBED)
            load_T(w2T, d_w2, HID, HID)
            load_T(w3T, d_w3, OUTD, HID)
            for l in range(LAYERS):
                for d in range(2):
                    load_T(wihT[l][d], d_wih[l, d], 3 * H, HID)
                    # Whh: [384, 128] -> single column block, but split by gate
                    # into three [128,128] stationaries.
                    for g in range(3):
                        tmp = wp.tile([128, 128], F32, tag="wtmp")
                        nc.sync.dma_start(out=tmp[:, :],
                                          in_=d_whh[l, d][g * H:(g + 1) * H, :])
                        ps = wps.tile([128, 128], F32, tag="wps")
                        mm_transpose(ps[:, :], tmp[:, :], 128, 128)
                        nc.vector.tensor_copy(out=whhT[l][d][g][:, :], in_=ps[:, :])

            # ---------- bias prep ----------
            # stage all bias vectors on partition 0, then "columnize" each
            # 128-chunk to a [128,1] tile via a K=1 matmul with ones.
            stages = {}
            for name, ap_, n in (("b1", d_b1, HID), ("b2", d_b2, HID),
                                 ("b3", d_b3, OUTD),
                                 ("bih", d_bih.flatten(), LAYERS * 2 * 3 * H),
                                 ("bhh", d_bhh.flatten(), LAYERS * 2 * 3 * H)):
                st = bp.tile([1, n], F32, tag=f"bstage_{name}")
                nc.sync.dma_start(out=st[0:1, :], in_=ap_.unsqueeze(0))
                stages[name] = st

            nc.vector.tensor_copy(out=b3_row[0:1, :OUTD], in_=stages["b3"][0:1, :OUTD])

            def columnize(dst, stage_name, src_off, n=128):
                stage = stages[stage_name]
                ps = wps.tile([128, 1], F32, tag="bcol")
                nc.tensor.matmul(out=ps[:n, :], lhsT=stage[0:1, src_off:src_off + n],
                                 rhs=ones_row[0:1, 0:1], start=True, stop=True)
                nc.vector.tensor_copy(out=dst[:n, :], in_=ps[:n, :])

            for m in range(2):
                columnize(b1c[m], "b1", m * 128)
                columnize(b2c[m], "b2", m * 128)
            for l in range(LAYERS):
                for d in range(2):
                    base = (l * 2 + d) * 3 * H
                    for g in range(2):  # r, z -> folded bih+bhh
                        tmp_a = bp.tile([128, 1], F32, tag="btmp_a")
                        tmp_b = bp.tile([128, 1], F32, tag="btmp_b")
                        ps = wps.tile([128, 1], F32, tag="bcol")
                        nc.tensor.matmul(out=ps[:, :],
                                         lhsT=stages["bih"][0:1, base + g * H:base + (g + 1) * H],
                                         rhs=ones_row[0:1, 0:1], start=True, stop=True)
                        nc.vector.tensor_copy(out=tmp_a[:, :], in_=ps[:, :])
                        ps2 = wps.tile([128, 1], F32, tag="bcol")
                        nc.tensor.matmul(out=ps2[:, :],
                                         lhsT=stages["bhh"][0:1, base + g * H:base + (g + 1) * H],
                                         rhs=ones_row[0:1, 0:1], start=True, stop=True)
                        nc.vector.tensor_copy(out=tmp_b[:, :], in_=ps2[:, :])
                        nc.vector.tensor_add(out=brz[l][d][g][:, :],
                                             in0=tmp_a[:, :], in1=tmp_b[:, :])
                    columnize(bihn[l][d], "bih", base + 2 * H)
                    columnize(bhhn[l][d], "bhh", base + 2 * H)

        # ---------- embedding gather + transpose + W1 ----------
        if LVL >= 1:
          with tc.tile_pool(name="emb", bufs=6) as ep, \
             tc.tile_pool(name="emb_ps", bufs=4, space="PSUM") as eps:
            for g in range(G):
                et = ep.tile([128, EMBED], F32, tag="etile")
                nc.gpsimd.indirect_dma_start(
                    out=et[:, :], out_offset=None, in_=d_embed[:, :],
                    in_offset=IndirectOffsetOnAxis(ap=idx_sb[:, g:g + 1], axis=0))
                for fb in range(2):
                    ps = eps.tile([128, 128], F32, tag="eps")
                    nc.tensor.matmul(out=ps[:, :], lhsT=et[:, fb * 128:(fb + 1) * 128],
                                     rhs=ident[:, :], start=True, stop=True)
                    nc.vector.tensor_copy(out=ET[fb][:, g * 128:(g + 1) * 128],
                                          in_=ps[:, :])

            # h1T[m] = W1[m-block] @ ET + b1   (feature-major)
            for m in range(2):
                for n in range(NNT):
                    ps = eps.tile([128, NT], F32, tag="mmps")
                    for k in range(2):
                        nc.tensor.matmul(
                            out=ps[:, :],
                            lhsT=_cast(w1T[k][:, m * 128:(m + 1) * 128], BULK_F32R),
                            rhs=_cast(ET[k][:, n * NT:(n + 1) * NT], BULK_F32R),
                            start=(k == 0), stop=(k == 1))
                    nc.scalar.activation(out=hT[m][:, n * NT:(n + 1) * NT], in_=ps[:, :],
                                         func=AF.Identity, bias=b1c[m][:, :], scale=1.0)

        # ---------- layers ----------
        for l in range(LAYERS if LVL >= 2 else 0):
            # input projections xp[dir][gate] = Wih_ld[gate] @ hT (+ biases)
            with tc.tile_pool(name=f"xp{l}_ps", bufs=6, space="PSUM") as xps:
                for d in range(2):
                    xp = xpF if d == 0 else xpB
                    for m in range(3):
                        bias = brz[l][d][m] if m < 2 else bihn[l][d]
                        for n in range(NNT):
                            ps = xps.tile([128, NT], F32, tag="xpps")
                            for k in range(2):
                                nc.tensor.matmul(
                                    out=ps[:, :],
                                    lhsT=_cast(wihT[l][d][k][:, m * 128:(m + 1) * 128], BULK_F32R),
                                    rhs=_cast(hT[k][:, n * NT:(n + 1) * NT], BULK_F32R),
                                    start=(k == 0), stop=(k == 1))
                            nc.scalar.activation(out=xp[m][:, n * NT:(n + 1) * NT],
                                                 in_=ps[:, :], func=AF.Identity,
                                                 bias=bias[:, :], scale=1.0)

            # bhh_n broadcast stage for the n-gate PSUM preload
            nc.vector.tensor_scalar_add(out=bhh_n_stage[:, 0:BC], in0=zeros_bc[:, :],
                                        scalar1=bhhn[l][0][:, :])
            nc.vector.tensor_scalar_add(out=bhh_n_stage[:, BC:2 * BC], in0=zeros_bc[:, :],
                                        scalar1=bhhn[l][1][:, :])

            # ---------- the scan ----------
            W = whhT[l]
            with tc.tile_pool(name=f"scan{l}_rz", bufs=3, space="PSUM") as prz, \
                 tc.tile_pool(name=f"scan{l}_n", bufs=3, space="PSUM") as pn, \
                 tc.tile_pool(name=f"scan{l}_sb", bufs=3) as psb:
                for t in range(S if LVL >= 3 else 0):
                    u = S - 1 - t  # bwd position
                    hF = outF[:, (t - 1) * BC:t * BC] if t > 0 else zeros_bc[:, :]
                    hB = outB[:, (u + 1) * BC:(u + 2) * BC] if t > 0 else zeros_bc[:, :]
                    hFf = hF.bitcast(F32)
                    hBf = hB.bitcast(F32)

                    rz = prz.tile([128, 4 * BC], F32, tag="rz")
                    hn = pn.tile([128, 2 * BC], F32, tag="hn")

                    # PSUM preload of xp (+bhh_n) via identity matmuls
                    idw = ident_r[:, :]
                    nc.tensor.matmul(out=rz[:, 0:BC], lhsT=idw,
                                     rhs=_cast(xpF[0][:, t * BC:(t + 1) * BC], SCAN_F32R),
                                     start=True, stop=False, skip_group_check=True)
                    nc.tensor.matmul(out=rz[:, BC:2 * BC], lhsT=idw,
                                     rhs=_cast(xpB[0][:, u * BC:(u + 1) * BC], SCAN_F32R),
                                     start=False, stop=False, skip_group_check=True)
                    nc.tensor.matmul(out=rz[:, 2 * BC:3 * BC], lhsT=idw,
                                     rhs=_cast(xpF[1][:, t * BC:(t + 1) * BC], SCAN_F32R),
                                     start=False, stop=False, skip_group_check=True)
                    nc.tensor.matmul(out=rz[:, 3 * BC:4 * BC], lhsT=idw,
                                     rhs=_cast(xpB[1][:, u * BC:(u + 1) * BC], SCAN_F32R),
                                     start=False, stop=False, skip_group_check=True)
                    nc.tensor.matmul(out=hn[:, :], lhsT=idw,
                                     rhs=_cast(bhh_n_stage[:, :], SCAN_F32R),
                                     start=True, stop=False, skip_group_check=True)

                    # gate matmuls accumulate Whh @ h
                    hFc = _cast(hF, SCAN_F32R)
                    hBc = _cast(hB, SCAN_F32R)
                    nc.tensor.matmul(out=rz[:, 0:BC], lhsT=_cast(W[0][0][:, :], SCAN_F32R),
                                     rhs=hFc, start=False, stop=False, skip_group_check=True)
                    nc.tensor.matmul(out=rz[:, BC:2 * BC], lhsT=_cast(W[1][0][:, :], SCAN_F32R),
                                     rhs=hBc, start=False, stop=False, skip_group_check=True)
                    nc.tensor.matmul(out=rz[:, 2 * BC:3 * BC], lhsT=_cast(W[0][1][:, :], SCAN_F32R),
                                     rhs=hFc, start=False, stop=True, skip_group_check=True)
                    nc.tensor.matmul(out=rz[:, 3 * BC:4 * BC], lhsT=_cast(W[1][1][:, :], SCAN_F32R),
                                     rhs=hBc, start=False, stop=True, skip_group_check=True)
                    nc.tensor.matmul(out=hn[:, 0:BC], lhsT=_cast(W[0][2][:, :], SCAN_F32R),
                                     rhs=hFc, start=False, stop=False, skip_group_check=True)
                    nc.tensor.matmul(out=hn[:, BC:2 * BC], lhsT=_cast(W[1][2][:, :], SCAN_F32R),
                                     rhs=hBc, start=False, stop=True, skip_group_check=True)

                    # sigmoid over r|z for both dirs, then gate algebra
                    rz_sb = psb.tile([128, 4 * BC], F32, tag="rz_sb")
                    nc.scalar.activation(out=rz_sb[:, :], in_=rz[:, :], func=AF.Sigmoid)

                    t1 = psb.tile([128, 2 * BC], F32, tag="t1")
                    nc.vector.tensor_mul(out=t1[:, :], in0=rz_sb[:, 0:2 * BC], in1=hn[:, :])
                    npre = psb.tile([128, 2 * BC], F32, tag="npre")
                    nc.vector.tensor_add(out=npre[:, 0:BC], in0=t1[:, 0:BC],
                                         in1=xpF[2][:, t * BC:(t + 1) * BC].bitcast(F32))
                    nc.vector.tensor_add(out=npre[:, BC:2 * BC], in0=t1[:, BC:2 * BC],
                                         in1=xpB[2][:, u * BC:(u + 1) * BC].bitcast(F32))

                    omz = psb.tile([128, 2 * BC], F32, tag="omz")
                    nc.vector.tensor_scalar(out=omz[:, :], in0=rz_sb[:, 2 * BC:4 * BC],
                                            scalar1=-1.0, scalar2=1.0,
                                            op0=ALU.mult, op1=ALU.add)
                    p = psb.tile([128, 2 * BC], F32, tag="p")
                    nc.vector.tensor_mul(out=p[:, 0:BC], in0=rz_sb[:, 2 * BC:3 * BC], in1=hFf)
                    nc.vector.tensor_mul(out=p[:, BC:2 * BC], in0=rz_sb[:, 3 * BC:4 * BC], in1=hBf)

                    n_sb = psb.tile([128, 2 * BC], F32, tag="n_sb")
                    nc.scalar.activation(out=n_sb[:, :], in_=npre[:, :], func=AF.Tanh)

                    t2 = psb.tile([128, 2 * BC], F32, tag="t2")
                    nc.vector.tensor_mul(out=t2[:, :], in0=omz[:, :], in1=n_sb[:, :])
                    nc.vector.tensor_add(out=outF[:, t * BC:(t + 1) * BC],
                                         in0=t2[:, 0:BC], in1=p[:, 0:BC])
                    nc.vector.tensor_add(out=outB[:, u * BC:(u + 1) * BC],
                                         in0=t2[:, BC:2 * BC], in1=p[:, BC:2 * BC])

            # residual + concat: hT <- [outF, outB] + hT
            CH = 2048 if NTOK % 2048 == 0 else NTOK
            for c in range(NTOK // CH if LVL >= 3 else 0):
                sl = slice(c * CH, (c + 1) * CH)
                nc.vector.tensor_add(out=hT[0][:, sl], in0=hT[0][:, sl], in1=outF[:, sl])
                nc.vector.tensor_add(out=hT[1][:, sl], in0=hT[1][:, sl], in1=outB[:, sl])

        # ---------- tail: W2 + gelu, W3 + b3, output ----------
        if LVL >= 4:
          with tc.tile_pool(name="tail", bufs=4) as tp, \
             tc.tile_pool(name="tail_ps", bufs=4, space="PSUM") as tps:
            for m in range(2):
                for n in range(NNT):
                    ps = tps.tile([128, NT], F32, tag="w2ps")
                    for k in range(2):
                        nc.tensor.matmul(
                            out=ps[:, :],
                            lhsT=_cast(w2T[k][:, m * 128:(m + 1) * 128], BULK_F32R),
                            rhs=_cast(hT[k][:, n * NT:(n + 1) * NT], BULK_F32R),
                            start=(k == 0), stop=(k == 1))
                    nc.scalar.activation(out=out2T[m][:, n * NT:(n + 1) * NT], in_=ps[:, :],
                                         func=gelu, bias=b2c[m][:, :], scale=1.0)

            for mt in range(NTOK // 128):
                ps = tps.tile([128, OUTD], F32, tag="w3ps")
                # bias via K=1 ones matmul, then accumulate the two k-blocks
                nc.tensor.matmul(out=ps[:, :], lhsT=ones_row[0:1, :],
                                 rhs=b3_row[0:1, :OUTD], start=True, stop=False,
                                 skip_group_check=True)
                for k in range(2):
                    nc.tensor.matmul(
                        out=ps[:, :],
                        lhsT=_cast(out2T[k][:, mt * 128:(mt + 1) * 128], BULK_F32R),
                        rhs=_cast(w3T[k][:, :OUTD], BULK_F32R),
                        start=False, stop=(k == 1), skip_group_check=True)
                yt = tp.tile([128, OUTD], F32, tag="ytile")
                nc.vector.tensor_copy(out=yt[:, :], in_=ps[:, :])
                nc.sync.dma_start(out=d_y[mt * 128:(mt + 1) * 128, :], in_=yt[:, :])

    nc.compile()
    return nc


# ---------------- host wrapper ----------------

_NC_CACHE = {}


def _get_nc(S):
    if S not in _NC_CACHE:
        _NC_CACHE[S] = build_nc(S)
    return _NC_CACHE[S]


def make_in_maps(x, embed, W1, b1, gru_Wih, gru_Whh, gru_bih, gru_bhh,
                 W2, b2, W3, b3, S=S_FULL):
    f = lambda a: np.ascontiguousarray(np.asarray(a), dtype=np.float32)
    x = np.asarray(x)
    G = BC * S // 128
    common = dict(embed=f(embed), W1=f(W1), b1=f(b1), gru_Wih=f(gru_Wih),
                  gru_Whh=f(gru_Whh), gru_bih=f(gru_bih), gru_bhh=f(gru_bhh),
                  W2=f(W2), b2=f(b2), W3=f(W3), b3=f(b3))
    in_maps = []
    for c in range(NCORES):
        xs = x[c * BC:(c + 1) * BC, :S].astype(np.int32)  # [BC, S]
        toks = np.ascontiguousarray(xs.T).reshape(-1)  # time-major [S*BC]
        x_cols = np.ascontiguousarray(toks.reshape(G, 128).T)  # [128, G]
        in_maps.append(dict(x_cols=x_cols, **common))
    return in_maps


def run(inputs, S=S_FULL, trace=False):
    nc = _get_nc(S)
    in_maps = make_in_maps(**inputs, S=S)
    res = bass_utils.run_bass_kernel_spmd(nc, in_maps, core_ids=list(range(NCORES)),
                                          trace=trace)
    outs = []
    for c in range(NCORES):
        y_tm = res.results[c]["y"]  # [S*BC, OUTD] time-major
        outs.append(y_tm.reshape(S, BC, OUTD).transpose(1, 0, 2))
    return np.concatenate(outs, axis=0), res


def kernel(**inputs):
    out, _ = run(inputs, S=S_FULL, trace=False)
    return out


# revision 29
# speedup vs baseline: 1.6724x; 1.0212x over previous
"""Bass/Trainium2 kernel for the bidirectional 2-layer GRU PoS model.

Sharding: data-parallel over batch. Each of the 8 cores processes 8 of the
64 sequences end-to-end (embed gather -> W1 -> 2x BiGRU -> W2+gelu -> W3).
Weights are replicated; no collectives.

Layout convention on-chip: "transposed" / feature-on-partitions. Tokens are
ordered time-major: token column = t*BC + b. This makes the per-timestep
scan slices contiguous ([128, BC] blocks) and lets the GRU state feed the
recurrent matmuls (contraction over the hidden dim = partitions) directly.
"""

import os

os.environ.setdefault("MYCRO_LOCAL_CACHE", "1")

import numpy as np

import concourse.bass as bass
import concourse.mybir as mybir
import concourse.tile as tile
from concourse import bacc
from concourse.bass import IndirectOffsetOnAxis
from concourse import bass_utils
from concourse.masks import make_identity

F32 = mybir.dt.float32
F32R = mybir.dt.float32r
I32 = mybir.dt.int32
AF = mybir.ActivationFunctionType
ALU = mybir.AluOpType

VOCAB, EMBED, HID, OUTD = 50000, 256, 256, 50
LAYERS = 2
B_FULL, S_FULL = 64, 512
NCORES = 8
BC = B_FULL // NCORES  # 8 sequences per core
H = HID // 2  # 128 per-direction hidden

# dtype knobs: f32r = native single-pass fp32 on the PE (fast), f32 = 2-pass.
BULK_F32R = True
SCAN_F32R = True


def _cast(ap, on):
    return ap.bitcast(F32R) if on else ap


def build_nc(S=S_FULL, gelu=AF.Gelu, upto="all"):
    LEVELS = {"prep": 0, "embed": 1, "xp": 2, "scan": 3, "all": 4}
    LVL = LEVELS[upto]
    NTOK = BC * S
    G = NTOK // 128  # token tiles for embedding gather
    NT = 512 if NTOK % 512 == 0 else NTOK  # matmul n-tile (tokens)
    NNT = NTOK // NT

    nc = bacc.Bacc("TRN2", target_bir_lowering=False, debug=False,
                   num_devices=NCORES)

    # ---- DRAM I/O ----
    d_xcols = nc.dram_tensor("x_cols", [128, G], I32, kind="ExternalInput").ap()
    d_embed = nc.dram_tensor("embed", [VOCAB, EMBED], F32, kind="ExternalInput").ap()
    d_w1 = nc.dram_tensor("W1", [HID, EMBED], F32, kind="ExternalInput").ap()
    d_b1 = nc.dram_tensor("b1", [HID], F32, kind="ExternalInput").ap()
    d_wih = nc.dram_tensor("gru_Wih", [LAYERS, 2, 3 * H, HID], F32, kind="ExternalInput").ap()
    d_whh = nc.dram_tensor("gru_Whh", [LAYERS, 2, 3 * H, H], F32, kind="ExternalInput").ap()
    d_bih = nc.dram_tensor("gru_bih", [LAYERS, 2, 3 * H], F32, kind="ExternalInput").ap()
    d_bhh = nc.dram_tensor("gru_bhh", [LAYERS, 2, 3 * H], F32, kind="ExternalInput").ap()
    d_w2 = nc.dram_tensor("W2", [HID, HID], F32, kind="ExternalInput").ap()
    d_b2 = nc.dram_tensor("b2", [HID], F32, kind="ExternalInput").ap()
    d_w3 = nc.dram_tensor("W3", [OUTD, HID], F32, kind="ExternalInput").ap()
    d_b3 = nc.dram_tensor("b3", [OUTD], F32, kind="ExternalInput").ap()
    d_y = nc.dram_tensor("y", [NTOK, OUTD], F32, kind="ExternalOutput").ap()

    # ---- persistent SBUF ----
    def sb(name, shape, dt=F32):
        return nc.alloc_sbuf_tensor(name, list(shape), dt).ap()

    # big SBUF arena (f32r):
    #  hT: current layer input (transposed feature-major, position order)
    #  A_rz: scan-ordered input projections, 4*BC cols per round (rF|rB|zF|zB)
    #        (bwd dir stored round-ordered = time-reversed)
    #  A_n:  scan-ordered n-gate input projections, 2*BC per round (nF|nB)
    #  A_oF/A_oB: scan outputs (fwd position order / bwd round order);
    #             reused as ET (embedding transposed) before the layers.
    #  out2T carved from A_rz at the tail.
    hT = [sb(f"a_hT{i}", [128, NTOK], F32R) for i in range(2)]
    A_rz = sb("a_rz", [128, 4 * NTOK], F32R)
    A_n = sb("a_n", [128, 2 * NTOK], F32R)
    A_oF = sb("a_oF", [128, NTOK], F32R)
    A_oB = sb("a_oB", [128, NTOK], F32R)
    ET = [A_oF, A_oB]
    outF, outB = A_oF, A_oB
    out2T = [A_rz[:, 0:NTOK], A_rz[:, NTOK:2 * NTOK]]
    rz4 = A_rz.rearrange("p (s g b) -> p s g b", g=4, b=BC)
    n2 = A_n.rearrange("p (s g b) -> p s g b", g=2, b=BC)

    ident = sb("ident", [128, 128])
    ident_r = sb("ident_r", [128, 128], F32R)
    idx_sb = sb("idx", [128, G], I32)
    ones_row = sb("ones_row", [1, 128])
    b3_row = sb("b3_row", [1, 64])
    zeros_bc = sb("zeros_bc", [128, BC], F32R)
    bhh_n_stage = sb("bhh_n_stage", [128, 2 * BC], F32R)

    # transposed weights
    w1T = [sb(f"w1T{k}", [128, HID], F32R) for k in range(2)]
    w2T = [sb(f"w2T{k}", [128, HID], F32R) for k in range(2)]
    w3T = [sb(f"w3T{k}", [128, OUTD], F32R) for k in range(2)]
    wihT = [[[sb(f"wihT_{l}_{d}_{k}", [128, 3 * H], F32R) for k in range(2)]
             for d in range(2)] for l in range(LAYERS)]
    whhT = [[[sb(f"whhT_{l}_{d}_{g}", [128, H], F32R) for g in range(3)]
             for d in range(2)] for l in range(LAYERS)]

    # per-partition bias columns [128, 1]
    b1c = [sb(f"b1c{m}", [128, 1]) for m in range(2)]
    b2c = [sb(f"b2c{m}", [128, 1]) for m in range(2)]
    # per (layer, dir): rz-folded (bih+bhh) for gates r,z ; bih_n ; bhh_n
    brz = [[[sb(f"brz_{l}_{d}_{g}", [128, 1]) for g in range(2)]
            for d in range(2)] for l in range(LAYERS)]
    bihn = [[sb(f"bihn_{l}_{d}", [128, 1]) for d in range(2)] for l in range(LAYERS)]
    bhhn = [[sb(f"bhhn_{l}_{d}", [128, 1]) for d in range(2)] for l in range(LAYERS)]

    with tile.TileContext(nc) as tc:
        # all gpsimd-engine prep first, then one PE op consuming ident so
        # later transpose-matmuls (which have a single sync-wait slot) only
        # ever need to wait on their DMA.
        nc.gpsimd.memset(ones_row[:, :], 1.0)
        nc.gpsimd.memset(zeros_bc[:, :].bitcast(F32), 0.0)
        nc.gpsimd.memset(b3_row[:, :], 0.0)
        make_identity(nc, ident[:, :])
        nc.vector.tensor_copy(out=ident_r[:, :], in_=ident[:, :])
        nc.sync.dma_start(out=idx_sb[:, :], in_=d_xcols[:, :])

        # ---------- weight prep ----------
        with tc.tile_pool(name="wprep", bufs=8) as wp, \
             tc.tile_pool(name="bstage", bufs=1) as bp, \
             tc.tile_pool(name="wprep_ps", bufs=4, space="PSUM") as wps:
            # transpose via a NORMAL matmul (out = lhsT.T @ I): the
            # is_transpose path only has one sync-wait slot in walrus codegen.
            def mm_transpose(out, in_, rsz, csz):
                nc.tensor.matmul(out=out, lhsT=in_, rhs=ident[:rsz, :rsz],
                                 start=True, stop=True)

            dummy_ps = wps.tile([128, 128], F32, tag="wps")
            mm_transpose(dummy_ps[:, :], ident[:, :], 128, 128)

            def load_T(dst_tiles, wa, R, C):
                # dst_tiles[cb][0:csz, 0:R] = wa.T block-columns
                for cb in range(len(dst_tiles)):
                    csz = min(128, C - cb * 128)
                    for rb in range((R + 127) // 128):
                        rsz = min(128, R - rb * 128)
                        tmp = wp.tile([128, 128], F32, tag="wtmp")
                        nc.sync.dma_start(
                            out=tmp[:rsz, :csz],
                            in_=wa[rb * 128:rb * 128 + rsz, cb * 128:cb * 128 + csz])
                        ps = wps.tile([128, 128], F32, tag="wps")
                        mm_transpose(ps[:csz, :rsz], tmp[:rsz, :csz], rsz, csz)
                        nc.vector.tensor_copy(
                            out=dst_tiles[cb][:csz, rb * 128:rb * 128 + rsz],
                            in_=ps[:csz, :rsz])

            load_T(w1T, d_w1, HID, EMBED)
            load_T(w2T, d_w2, HID, HID)
            load_T(w3T, d_w3, OUTD, HID)
            for l in range(LAYERS):
                for d in range(2):
                    load_T(wihT[l][d], d_wih[l, d], 3 * H, HID)
                    # Whh: [384, 128] -> single column block, but split by gate
                    # into three [128,128] stationaries.
                    for g in range(3):
                        tmp = wp.tile([128, 128], F32, tag="wtmp")
                        nc.sync.dma_start(out=tmp[:, :],
                                          in_=d_whh[l, d][g * H:(g + 1) * H, :])
                        ps = wps.tile([128, 128], F32, tag="wps")
                        mm_transpose(ps[:, :], tmp[:, :], 128, 128)
                        nc.vector.tensor_copy(out=whhT[l][d][g][:, :], in_=ps[:, :])

            # ---------- bias prep ----------
            # stage all bias vectors on partition 0, then "columnize" each
            # 128-chunk to a [128,1] tile via a K=1 matmul with ones.
            stages = {}
            for name, ap_, n in (("b1", d_b1, HID), ("b2", d_b2, HID),
                                 ("b3", d_b3, OUTD),
                                 ("bih", d_bih.flatten(), LAYERS * 2 * 3 * H),
                                 ("bhh", d_bhh.flatten(), LAYERS * 2 * 3 * H)):
                st = bp.tile([1, n], F32, tag=f"bstage_{name}")
                nc.sync.dma_start(out=st[0:1, :], in_=ap_.unsqueeze(0))
                stages[name] = st

            nc.vector.tensor_copy(out=b3_row[0:1, :OUTD], in_=stages["b3"][0:1, :OUTD])

            def columnize(dst, stage_name, src_off, n=128):
                stage = stages[stage_name]
                ps = wps.tile([128, 1], F32, tag="bcol")
                nc.tensor.matmul(out=ps[:n, :], lhsT=stage[0:1, src_off:src_off + n],
                                 rhs=ones_row[0:1, 0:1], start=True, stop=True)
                nc.vector.tensor_copy(out=dst[:n, :], in_=ps[:n, :])

            for m in range(2):
                columnize(b1c[m], "b1", m * 128)
                columnize(b2c[m], "b2", m * 128)
            for l in range(LAYERS):
                for d in range(2):
                    base = (l * 2 + d) * 3 * H
                    for g in range(2):  # r, z -> folded bih+bhh
                        tmp_a = bp.tile([128, 1], F32, tag="btmp_a")
                        tmp_b = bp.tile([128, 1], F32, tag="btmp_b")
                        ps = wps.tile([128, 1], F32, tag="bcol")
                        nc.tensor.matmul(out=ps[:, :],
                                         lhsT=stages["bih"][0:1, base + g * H:base + (g + 1) * H],
                                         rhs=ones_row[0:1, 0:1], start=True, stop=True)
                        nc.vector.tensor_copy(out=tmp_a[:, :], in_=ps[:, :])
                        ps2 = wps.tile([128, 1], F32, tag="bcol")
                        nc.tensor.matmul(out=ps2[:, :],
                                         lhsT=stages["bhh"][0:1, base + g * H:base + (g + 1) * H],
                                         rhs=ones_row[0:1, 0:1], start=True, stop=True)
                        nc.vector.tensor_copy(out=tmp_b[:, :], in_=ps2[:, :])
                        nc.vector.tensor_add(out=brz[l][d][g][:, :],
                                             in0=tmp_a[:, :], in1=tmp_b[:, :])
                    columnize(bihn[l][d], "bih", base + 2 * H)
                    columnize(bhhn[l][d], "bhh", base + 2 * H)

        # ---------- embedding gather + transpose + W1 ----------
        if LVL >= 1:
          with tc.tile_pool(name="emb", bufs=6) as ep, \
             tc.tile_pool(name="emb_ps", bufs=4, space="PSUM") as eps:
            for g in range(G):
                et = ep.tile([128, EMBED], F32, tag="etile")
                nc.gpsimd.indirect_dma_start(
                    out=et[:, :], out_offset=None, in_=d_embed[:, :],
                    in_offset=IndirectOffsetOnAxis(ap=idx_sb[:, g:g + 1], axis=0))
                for fb in range(2):
                    ps = eps.tile([128, 128], F32, tag="eps")
                    nc.tensor.matmul(out=ps[:, :], lhsT=et[:, fb * 128:(fb + 1) * 128],
                                     rhs=ident[:, :], start=True, stop=True)
                    nc.vector.tensor_copy(out=ET[fb][:, g * 128:(g + 1) * 128],
                                          in_=ps[:, :])

            # h1T[m] = W1[m-block] @ ET + b1   (feature-major)
            for m in range(2):
                for n in range(NNT):
                    ps = eps.tile([128, NT], F32, tag="mmps")
                    for k in range(2):
                        nc.tensor.matmul(
                            out=ps[:, :],
                            lhsT=_cast(w1T[k][:, m * 128:(m + 1) * 128], BULK_F32R),
                            rhs=_cast(ET[k][:, n * NT:(n + 1) * NT], BULK_F32R),
                            start=(k == 0), stop=(k == 1))
                    nc.scalar.activation(out=hT[m][:, n * NT:(n + 1) * NT], in_=ps[:, :],
                                         func=AF.Identity, bias=b1c[m][:, :], scale=1.0)

        # ---------- layers ----------
        RT = NT // BC  # rounds per n-tile
        for l in range(LAYERS if LVL >= 2 else 0):
            # input projections xp[dir][gate] = Wih_ld[gate] @ hT (+ biases),
            # written into scan-ordered buffers; bwd dir time-reversed via a
            # negative-stride destination AP.
            with tc.tile_pool(name=f"xp{l}_ps", bufs=6, space="PSUM") as xps:
                for d in range(2):
                    for m in range(3):
                        bias = brz[l][d][m] if m < 2 else bihn[l][d]
                        for n in range(NNT):
                            ps = xps.tile([128, NT], F32, tag="xpps")
                            for k in range(2):
                                nc.tensor.matmul(
                                    out=ps[:, :],
                                    lhsT=_cast(wihT[l][d][k][:, m * 128:(m + 1) * 128], BULK_F32R),
                                    rhs=_cast(hT[k][:, n * NT:(n + 1) * NT], BULK_F32R),
                                    start=(k == 0), stop=(k == 1))
                            # destination: rounds s = n*RT .. n*RT+RT-1 for fwd;
                            # s = S-1-n*RT .. S-1-n*RT-RT+1 (descending) for bwd
                            stop = S - 1 - (n + 1) * RT
                            rsl = slice(S - 1 - n * RT, None if stop < 0 else stop, -1)
                            if m < 2:
                                g = (0 if m == 0 else 2) + d
                                dst = (rz4[:, n * RT:(n + 1) * RT, g, :] if d == 0
                                       else rz4[:, rsl, g, :])
                            else:
                                dst = (n2[:, n * RT:(n + 1) * RT, 0, :] if d == 0
                                       else n2[:, rsl, 1, :])
                            nc.scalar.activation(out=dst, in_=ps[:, :], func=AF.Identity,
                                                 bias=bias[:, :], scale=1.0)

            # bhh_n broadcast stage for the n-gate PSUM preload
            nc.vector.tensor_scalar_add(out=bhh_n_stage[:, 0:BC], in0=zeros_bc[:, :],
                                        scalar1=bhhn[l][0][:, :])
            nc.vector.tensor_scalar_add(out=bhh_n_stage[:, BC:2 * BC], in0=zeros_bc[:, :],
                                        scalar1=bhhn[l][1][:, :])

            # ---------- the scan ----------
            W = whhT[l]
            with tc.tile_pool(name=f"scan{l}_rz", bufs=3, space="PSUM") as prz, \
                 tc.tile_pool(name=f"scan{l}_n", bufs=3, space="PSUM") as pn, \
                 tc.tile_pool(name=f"scan{l}_sb", bufs=3) as psb:
                for t in range(S if LVL >= 3 else 0):
                    hF = outF[:, (t - 1) * BC:t * BC] if t > 0 else zeros_bc[:, :]
                    hB = outB[:, (t - 1) * BC:t * BC] if t > 0 else zeros_bc[:, :]
                    hFf = hF.bitcast(F32)
                    hBf = hB.bitcast(F32)

                    rz = prz.tile([128, 4 * BC], F32, tag="rz")
                    hn = pn.tile([128, 2 * BC], F32, tag="hn")

                    # PSUM preload via identity matmuls: one for the packed
                    # rz block, one for the n-gate bhh broadcast.
                    nc.tensor.matmul(out=rz[:, :], lhsT=ident_r[:, :],
                                     rhs=A_rz[:, t * 4 * BC:(t + 1) * 4 * BC],
                                     start=True, stop=False, skip_group_check=True)
                    nc.tensor.matmul(out=hn[:, :], lhsT=ident_r[:, :],
                                     rhs=bhh_n_stage[:, :],
                                     start=True, stop=False, skip_group_check=True)

                    # gate matmuls accumulate Whh @ h  (rz cols: rF|rB|zF|zB)
                    nc.tensor.matmul(out=rz[:, 0:BC], lhsT=W[0][0][:, :],
                                     rhs=hF, start=False, stop=False, skip_group_check=True)
                    nc.tensor.matmul(out=rz[:, BC:2 * BC], lhsT=W[1][0][:, :],
                                     rhs=hB, start=False, stop=False, skip_group_check=True)
                    nc.tensor.matmul(out=rz[:, 2 * BC:3 * BC], lhsT=W[0][1][:, :],
                                     rhs=hF, start=False, stop=True, skip_group_check=True)
                    nc.tensor.matmul(out=rz[:, 3 * BC:4 * BC], lhsT=W[1][1][:, :],
                                     rhs=hB, start=False, stop=True, skip_group_check=True)
                    nc.tensor.matmul(out=hn[:, 0:BC], lhsT=W[0][2][:, :],
                                     rhs=hF, start=False, stop=False, skip_group_check=True)
                    nc.tensor.matmul(out=hn[:, BC:2 * BC], lhsT=W[1][2][:, :],
                                     rhs=hB, start=False, stop=True, skip_group_check=True)

                    # sigmoid over r|z for both dirs, then gate algebra
                    rz_sb = psb.tile([128, 4 * BC], F32, tag="rz_sb")
                    nc.scalar.activation(out=rz_sb[:, :], in_=rz[:, :], func=AF.Sigmoid)

                    t1 = psb.tile([128, 2 * BC], F32, tag="t1")
                    nc.vector.tensor_mul(out=t1[:, :], in0=rz_sb[:, 0:2 * BC], in1=hn[:, :])
                    npre = psb.tile([128, 2 * BC], F32, tag="npre")
                    nc.vector.tensor_add(out=npre[:, :], in0=t1[:, :],
                                         in1=A_n[:, t * 2 * BC:(t + 1) * 2 * BC].bitcast(F32))

                    omz = psb.tile([128, 2 * BC], F32, tag="omz")
                    nc.vector.tensor_scalar(out=omz[:, :], in0=rz_sb[:, 2 * BC:4 * BC],
                                            scalar1=-1.0, scalar2=1.0,
                                            op0=ALU.mult, op1=ALU.add)
                    p = psb.tile([128, 2 * BC], F32, tag="p")
                    nc.vector.tensor_mul(out=p[:, 0:BC], in0=rz_sb[:, 2 * BC:3 * BC], in1=hFf)
                    nc.vector.tensor_mul(out=p[:, BC:2 * BC], in0=rz_sb[:, 3 * BC:4 * BC], in1=hBf)

                    n_sb = psb.tile([128, 2 * BC], F32, tag="n_sb")
                    nc.scalar.activation(out=n_sb[:, :], in_=npre[:, :], func=AF.Tanh)

                    t2 = psb.tile([128, 2 * BC], F32, tag="t2")
                    nc.vector.tensor_mul(out=t2[:, :], in0=omz[:, :], in1=n_sb[:, :])
                    nc.vector.tensor_add(out=outF[:, t * BC:(t + 1) * BC],
                                         in0=t2[:, 0:BC], in1=p[:, 0:BC])
                    nc.vector.tensor_add(out=outB[:, t * BC:(t + 1) * BC],
                                         in0=t2[:, BC:2 * BC], in1=p[:, BC:2 * BC])

            # residual + concat: hT <- [outF, outB(reversed)] + hT
            oB3 = outB.rearrange("p (s b) -> p s b", b=BC)
            CH = 2048 if NTOK % 2048 == 0 else NTOK
            RCH = CH // BC
            for c in range(NTOK // CH if LVL >= 3 else 0):
                sl = slice(c * CH, (c + 1) * CH)
                nc.vector.tensor_add(out=hT[0][:, sl], in0=hT[0][:, sl], in1=outF[:, sl])
                rstop = S - 1 - (c + 1) * RCH
                rev = oB3[:, S - 1 - c * RCH:None if rstop < 0 else rstop:-1, :]
                nc.vector.tensor_add(out=hT[1][:, sl], in0=hT[1][:, sl], in1=rev)

        # ---------- tail: W2 + gelu, W3 + b3, output ----------
        if LVL >= 4:
          with tc.tile_pool(name="tail", bufs=4) as tp, \
             tc.tile_pool(name="tail_ps", bufs=4, space="PSUM") as tps:
            for m in range(2):
                for n in range(NNT):
                    ps = tps.tile([128, NT], F32, tag="w2ps")
                    for k in range(2):
                        nc.tensor.matmul(
                            out=ps[:, :],
                            lhsT=_cast(w2T[k][:, m * 128:(m + 1) * 128], BULK_F32R),
                            rhs=_cast(hT[k][:, n * NT:(n + 1) * NT], BULK_F32R),
                            start=(k == 0), stop=(k == 1))
                    nc.scalar.activation(out=out2T[m][:, n * NT:(n + 1) * NT], in_=ps[:, :],
                                         func=gelu, bias=b2c[m][:, :], scale=1.0)

            for mt in range(NTOK // 128):
                ps = tps.tile([128, OUTD], F32, tag="w3ps")
                # bias via K=1 ones matmul, then accumulate the two k-blocks
                nc.tensor.matmul(out=ps[:, :], lhsT=ones_row[0:1, :],
                                 rhs=b3_row[0:1, :OUTD], start=True, stop=False,
                                 skip_group_check=True)
                for k in range(2):
                    nc.tensor.matmul(
                        out=ps[:, :],
                        lhsT=_cast(out2T[k][:, mt * 128:(mt + 1) * 128], BULK_F32R),
                        rhs=_cast(w3T[k][:, :OUTD], BULK_F32R),
                        start=False, stop=(k == 1), skip_group_check=True)
                yt = tp.tile([128, OUTD], F32, tag="ytile")
                nc.vector.tensor_copy(out=yt[:, :], in_=ps[:, :])
                nc.sync.dma_start(out=d_y[mt * 128:(mt + 1) * 128, :], in_=yt[:, :])

    nc.compile()
    return nc


# ---------------- host wrapper ----------------

_NC_CACHE = {}


def _get_nc(S):
    if S not in _NC_CACHE:
        _NC_CACHE[S] = build_nc(S)
    return _NC_CACHE[S]


def make_in_maps(x, embed, W1, b1, gru_Wih, gru_Whh, gru_bih, gru_bhh,
                 W2, b2, W3, b3, S=S_FULL):
    f = lambda a: np.ascontiguousarray(np.asarray(a), dtype=np.float32)
    x = np.asarray(x)
    G = BC * S // 128
    common = dict(embed=f(embed), W1=f(W1), b1=f(b1), gru_Wih=f(gru_Wih),
                  gru_Whh=f(gru_Whh), gru_bih=f(gru_bih), gru_bhh=f(gru_bhh),
                  W2=f(W2), b2=f(b2), W3=f(W3), b3=f(b3))
    in_maps = []
    for c in range(NCORES):
        xs = x[c * BC:(c + 1) * BC, :S].astype(np.int32)  # [BC, S]
        toks = np.ascontiguousarray(xs.T).reshape(-1)  # time-major [S*BC]
        x_cols = np.ascontiguousarray(toks.reshape(G, 128).T)  # [128, G]
        in_maps.append(dict(x_cols=x_cols, **common))
    return in_maps


def run(inputs, S=S_FULL, trace=False):
    nc = _get_nc(S)
    in_maps = make_in_maps(**inputs, S=S)
    res = bass_utils.run_bass_kernel_spmd(nc, in_maps, core_ids=list(range(NCORES)),
                                          trace=trace)
    outs = []
    for c in range(NCORES):
        y_tm = res.results[c]["y"]  # [S*BC, OUTD] time-major
        outs.append(y_tm.reshape(S, BC, OUTD).transpose(1, 0, 2))
    return np.concatenate(outs, axis=0), res


def kernel(**inputs):
    out, _ = run(inputs, S=S_FULL, trace=False)
    return out
